# revision 14
# baseline (speedup 1.0000x reference)
"""EnhancedTemporalAttention Trainium2 kernel (v2).

Full module: GroupNorm(32) -> QKV 1x1conv -> 8-head attention (softmax) ->
out 1x1conv + bias -> +residual, on x [4, 512, 2048] fp32.

Sharding: 8 cores = (batch b = core//2) x (head-half hg = core%2).  Each
core computes GroupNorm stats + its 4 heads' Q/K/V over the full sequence,
attention for all 2048 queries, and a partial out-projection (contraction
over its 256 channels).  Host sums the two partials per batch and adds
residual + b_out exactly in fp32.

GroupNorm is folded into the QKV weights: w' = w * scale_c (per input
channel, scaled in place on Pool), plus a K=1 matvec for the bias term
which rides the projection PSUM->SBUF copies as a per-partition bias.

Attention uses transposed scores (keys on partitions, [k,q] layout); exp
splits between ACT (exact, 19/32 per pair) and DVE (Schraudolph int16
bit-trick into bf16 bits, 13/32).  AV runs with eT stationary / vT moving
so each matmul is only 65 output rows; softmax denominators ride a ones
column on vT; normalization is a per-partition multiply in [q,d] layout,
then a PE transpose (bf16) back to [c,q] for the out-projection, whose
PSUM result DMAs straight to DRAM.
"""
import sys

sys.path.insert(0, "/opt/trn_rl_repo")

import numpy as np
import ml_dtypes

import concourse.bacc as bacc
import concourse.bass as bass
import concourse.tile as tile
from concourse import mybir
from concourse.bass_utils import run_bass_kernel_spmd

F32 = mybir.dt.float32
F32R = mybir.dt.float32r
BF16 = mybir.dt.bfloat16
I16 = mybir.dt.int16

B = 4
C = 512
N = 2048
H = 8
HL = 4             # local heads per core
D = 64
G = 32             # groupnorm groups
CPG = C // G       # 16 channels per group
EPS = 1e-4
SCALE = D ** -0.5
NT = C // 128      # 4 input-channel tiles
MT = 2             # local qkv channel tiles (256 local channels)
NKB = N // 128     # 16 key blocks
QC = 4             # query chunks of 512
AF = mybir.ActivationFunctionType
ALU = mybir.AluOpType

# Schraudolph exp into bf16 bits: i16 = s*A_S + B_S, bitcast -> bf16
A_S = 184.6650085 * SCALE
B_S = 16249.1
# j's whose (whole-j, both-head) exp runs on ACT exactly; the rest run
# the Schraudolph bit-trick on DVE.
ACT_J = (0, 1, 3, 5, 7, 9, 11, 13, 15)


def _build(taps=False):
    nc = bacc.Bacc("TRN2", target_bir_lowering=False, debug=False)
    x_in = nc.dram_tensor("x", [C, N], BF16, kind="ExternalInput").ap()
    wqkvT_in = nc.dram_tensor("wqkvT", [C, 6 * 128], BF16,
                              kind="ExternalInput").ap()
    woutT_in = nc.dram_tensor("woutT", [MT * 128, C], BF16,
                              kind="ExternalInput").ap()
    gbo_in = nc.dram_tensor("gbo", [128, 8], F32, kind="ExternalInput").ap()
    gblk_in = nc.dram_tensor("gblk", [128, 8], F32, kind="ExternalInput").ap()
    gbt_in = nc.dram_tensor("gbt", [8, 128], F32, kind="ExternalInput").ap()
    id_in = nc.dram_tensor("ident", [128, 128], BF16,
                           kind="ExternalInput").ap()
    y_out = nc.dram_tensor("y", [C, N], F32, kind="ExternalOutput").ap()

    from contextlib import ExitStack
    with tile.TileContext(nc) as tc, ExitStack() as ctx:
        persist = ctx.enter_context(tc.tile_pool(name="persist", bufs=1))
        gn = ctx.enter_context(tc.tile_pool(name="gn", bufs=1))
        pspool = ctx.enter_context(tc.tile_pool(name="ps", bufs=1,
                                                space="PSUM"))
        expp = ctx.enter_context(tc.tile_pool(name="expp", bufs=1))
        drp = ctx.enter_context(tc.tile_pool(name="drp", bufs=1))

        # ---- persistent tiles ----
        X = [persist.tile([128, N], BF16, tag=f"X{t}", name=f"X{t}")
             for t in range(NT)]
        wT = [persist.tile([128, 6 * 128], BF16, tag=f"wT{kc}",
                           name=f"wT{kc}") for kc in range(NT)]
        woutT = [persist.tile([128, C], BF16, tag=f"woT{m}", name=f"woT{m}")
                 for m in range(MT)]
        q_sb = [persist.tile([128, N], BF16, tag=f"q{m}", name=f"q{m}")
                for m in range(MT)]
        k_sb = [persist.tile([128, N], BF16, tag=f"k{m}", name=f"k{m}")
                for m in range(MT)]
        vT_sb = [persist.tile([128, HL, 66], BF16, tag=f"vT{nb}",
                              name=f"vT{nb}") for nb in range(NKB)]
        ident = persist.tile([128, 128], BF16, tag="ident", name="ident")

        # PSUM rings: S 3x4KB + av 4KB = 16KB exactly; every other
        # psum user (GN, projections, transpose, out-proj) shares the
        # S ring, using a [:, 0:512] half-slot view.
        def new_S():
            return pspool.tile([128, 1024], F32, tag="S", name="S", bufs=3)

        def new_ops():
            return new_S()[:, 0:512]

        def new_av():
            return pspool.tile([128, 8, 128], F32, tag="av", name="av",
                               bufs=1)

        # ---- input loads ----
        # x: 8 chunks of [128,1024] on the HWDGE path (critical for stats);
        # misc/gbt woven in after chunk 3.  Weights + ident ride the
        # software DGE (gpsimd) so they bypass the serialized HWDGE device.
        gbo4 = gn.tile([128, 8], F32, tag="gbo4")
        gblk = gn.tile([128, 8], F32R, tag="gblk")
        gbt = gn.tile([8, 128], F32R, tag="gbt")
        for t in range(NT):
            for half in range(2):
                nc.sync.dma_start(
                    out=X[t][:, half * 1024:(half + 1) * 1024],
                    in_=x_in[t * 128:(t + 1) * 128,
                             half * 1024:(half + 1) * 1024])
        nc.sync.dma_start(out=gbo4, in_=gbo_in)
        nc.sync.dma_start(out=gblk, in_=gblk_in.bitcast(F32R))
        nc.sync.dma_start(out=gbt, in_=gbt_in.bitcast(F32R))
        for kc in range(NT):
            nc.sync.dma_start(out=wT[kc],
                              in_=wqkvT_in[kc * 128:(kc + 1) * 128, :])
        for m in range(MT):
            nc.sync.dma_start(out=woutT[m],
                              in_=woutT_in[m * 128:(m + 1) * 128, :])
        nc.sync.dma_start(out=ident, in_=id_in)
        gbo = [gbo4[:, 2 * t:2 * t + 2] for t in range(NT)]

        # ---- GroupNorm stats ----
        eps_t = gn.tile([G, 1], F32, tag="eps_t")
        nc.vector.memset(eps_t, EPS)
        sqw = gn.tile([G, 1], F32, tag="sqw")
        nc.scalar.activation(out=sqw, in_=eps_t, func=AF.Sqrt)
        mvv = []
        for t in range(NT):
            stats = gn.tile([128, 4, 6], F32, tag=f"st{t}", name=f"st{t}")
            for sg in range(4):
                nc.vector.bn_stats(out=stats[:, sg, :],
                                   in_=X[t][:, sg * 512:(sg + 1) * 512])
            mv = gn.tile([128, 2], F32, tag=f"mv{t}", name=f"mv{t}")
            nc.vector.bn_aggr(out=mv, in_=stats)
            mt = gn.tile([128, 2], F32R, tag=f"mvv{t}", name=f"mvv{t}")
            nc.vector.tensor_copy(mt[:, 0:1], mv[:, 0:1])
            nc.vector.scalar_tensor_tensor(
                out=mt[:, 1:2], in0=mv[:, 0:1], scalar=mv[:, 0:1],
                in1=mv[:, 1:2], op0=ALU.mult, op1=ALU.add)
            mvv.append(mt)
        g8ps = new_ops()
        for t in range(NT):
            nc.tensor.matmul(g8ps[0:8, t * 2:(t + 1) * 2],
                             lhsT=gblk, rhs=mvv[t],
                             start=(t == 0), stop=(t == NT - 1),
                             skip_group_check=True)
        g8 = gn.tile([8, NT, 2], F32, tag="g8")
        nc.vector.tensor_copy(g8.rearrange("p t s -> p (t s)"),
                              g8ps[0:8, 0:8])
        mean8 = gn.tile([8, NT], F32, tag="mean8")
        nc.vector.tensor_scalar_mul(mean8, g8[:, :, 0], 1.0 / CPG)
        ex28 = gn.tile([8, NT], F32, tag="ex28")
        nc.vector.tensor_scalar_mul(ex28, g8[:, :, 1], 1.0 / CPG)
        msq8 = gn.tile([8, NT], F32, tag="msq8")
        nc.vector.tensor_mul(msq8, mean8, mean8)
        var8 = gn.tile([8, NT], F32, tag="var8")
        nc.vector.tensor_tensor(out=var8, in0=ex28, in1=msq8,
                                op=ALU.subtract)
        std8 = gn.tile([8, NT], F32, tag="std8")
        nc.scalar.activation(out=std8, in_=var8, func=AF.Sqrt,
                             bias=eps_t[0:8, :])
        rstd8 = gn.tile([8, NT], F32, tag="rstd8")
        nc.vector.reciprocal(rstd8, std8)
        # preload the Exp table; chained after the real Sqrt via std8
        warm = gn.tile([8, NT], F32, tag="warm")
        nc.scalar.activation(out=warm, in_=std8, func=AF.Exp)
        mr8 = gn.tile([8, NT, 2], F32R, tag="mr8")
        nc.vector.tensor_copy(mr8[:, :, 0:1],
                              mean8.rearrange("p (t o) -> p t o", o=1))
        nc.vector.tensor_copy(mr8[:, :, 1:2],
                              rstd8.rearrange("p (t o) -> p t o", o=1))
        msps = new_ops()
        for t in range(NT):
            nc.tensor.matmul(msps[:, t * 2:(t + 1) * 2],
                             lhsT=gbt, rhs=mr8[:, t, :],
                             start=(t == 0), stop=(t == NT - 1),
                             skip_group_check=True)
        mscall = msps[:, 0:2 * NT].rearrange("p (t s) -> p t s", s=2)
        msb = gn.tile([128, NT, 2], F32, tag="msb")
        nc.vector.tensor_copy(msb.rearrange("p t s -> p (t s)"),
                              msps[:, 0:2 * NT])

        # per-channel scale_c = rstd*gamma; bias (bf16, for the matvec)
        # = beta - mean*scale_c, per-tile chains split DVE/Pool
        qkvb_ps = None
        scale_c = []
        for t in range(NT):
            eng = nc.vector if t % 2 == 0 else nc.gpsimd
            sc = gn.tile([128, 1], F32, tag=f"sc{t}", name=f"sc{t}")
            eng.tensor_mul(sc, msb[:, t, 1:2], gbo[t][:, 0:1])
            scale_c.append(sc)
            tmp = gn.tile([128, 1], F32, tag=f"tmp{t}", name=f"tmp{t}")
            eng.tensor_mul(tmp, msb[:, t, 0:1], sc)
            bb = gn.tile([128, 1], BF16, tag=f"bb{t}", name=f"bb{t}")
            eng.tensor_tensor(out=bb, in0=gbo[t][:, 1:2], in1=tmp,
                              op=ALU.subtract)
            # qkv bias matvec against RAW weights (before scaling)
            if qkvb_ps is None:
                qkvb_ps = new_ops()
            for oc in range(6):
                nc.tensor.matmul(qkvb_ps[:, 6 * t + oc:6 * t + oc + 1],
                                 lhsT=wT[t][:, oc * 128:(oc + 1) * 128],
                                 rhs=bb, start=True, stop=True,
                                 skip_group_check=True)
        # reduce the 4 kc partials: qkvb[:, oc] = sum_t qkvb_ps[:, 6t+oc]
        qkvb_all = gn.tile([128, NT, 6], F32, tag="qkvb_all")
        nc.vector.tensor_copy(qkvb_all.rearrange("p t s -> p (t s)"),
                              qkvb_ps[:, 0:24])
        qkvb01 = gn.tile([128, 6], F32, tag="qkvb01")
        nc.vector.tensor_tensor(out=qkvb01, in0=qkvb_all[:, 0, :],
                                in1=qkvb_all[:, 1, :], op=ALU.add)
        qkvb23 = gn.tile([128, 6], F32, tag="qkvb23")
        nc.vector.tensor_tensor(out=qkvb23, in0=qkvb_all[:, 2, :],
                                in1=qkvb_all[:, 3, :], op=ALU.add)
        qkvb = gn.tile([128, 6], F32, tag="qkvb")
        nc.vector.tensor_tensor(out=qkvb, in0=qkvb01, in1=qkvb23,
                                op=ALU.add)
        # scale weights in place (per input-channel partition)
        for t in range(NT):
            if t < 2:
                nc.scalar.activation(out=wT[t], in_=wT[t], func=AF.Copy,
                                     scale=scale_c[t])
            else:
                nc.gpsimd.tensor_scalar(out=wT[t], in0=wT[t],
                                        scalar1=scale_c[t], scalar2=None,
                                        op0=ALU.mult)

        # ---- projections ----
        def kq_proj(which, m, ncx, eng):
            ps = new_ops()
            col0 = (m if which == "q" else 2 + m) * 128
            for kc in range(NT):
                nc.tensor.matmul(
                    ps, lhsT=wT[kc][:, col0:col0 + 128],
                    rhs=X[kc][:, ncx * 512:(ncx + 1) * 512],
                    start=(kc == 0), stop=(kc == NT - 1))
            dst = (q_sb if which == "q" else k_sb)[m][
                :, ncx * 512:(ncx + 1) * 512]
            boff = (0 if which == "q" else 2) + m
            if eng == "act":
                nc.scalar.activation(out=dst, in_=ps, func=AF.Identity,
                                     bias=qkvb[:, boff:boff + 1])
            else:
                nc.vector.tensor_scalar(out=dst, in0=ps,
                                        scalar1=qkvb[:, boff:boff + 1],
                                        scalar2=None, op0=ALU.add)

        def v_proj(nb, eng):
            ps = new_ops()
            for kc in range(NT):
                nc.tensor.matmul(
                    ps[:, 0:256], lhsT=X[kc][:, nb * 128:(nb + 1) * 128],
                    rhs=wT[kc][:, 4 * 128:6 * 128],
                    start=(kc == 0), stop=(kc == NT - 1))
            src = ps[:, 0:256].rearrange("p (h d) -> p h d", h=HL)
            dst = vT_sb[nb][:, :, 0:64]
            # v bias is folded at the opair copy; here plain convert
            if eng == "act":
                nc.scalar.activation(out=dst, in_=src, func=AF.Copy)
            else:
                nc.vector.tensor_copy(dst, src)
            nc.gpsimd.memset(vT_sb[nb][:, :, 64:65], 1.0)

        engs = ["act", "dve"]
        for m in range(MT):
            for ncx in range(4):
                kq_proj("k", m, ncx, engs[(m * 4 + ncx) % 2])
        for m in range(MT):
            for ncx in range(4):
                kq_proj("q", m, ncx, engs[(m * 4 + ncx + 1) % 2])
        for nb in range(NKB):
            v_proj(nb, engs[nb % 2])

        # ---- attention ----
        pairs = [(qc, m) for qc in range(QC) for m in range(MT)]
        opair = {}

        def emit_scores(qc, m, j):
            s = new_S()
            nc.tensor.matmul(
                s[:, 0:512], lhsT=k_sb[m][0:64, j * 128:(j + 1) * 128],
                rhs=q_sb[m][0:64, qc * 512:(qc + 1) * 512],
                start=True, stop=True, tile_position=(0, 0),
                skip_group_check=True)
            nc.tensor.matmul(
                s[:, 512:1024], lhsT=k_sb[m][64:128, j * 128:(j + 1) * 128],
                rhs=q_sb[m][64:128, qc * 512:(qc + 1) * 512],
                start=True, stop=True, tile_position=(64, 0),
                skip_group_check=True)
            return s

        def emit_exp(s, eT, j):
            if j in ACT_J:
                nc.scalar.activation(out=eT.bitcast(BF16), in_=s,
                                     func=AF.Exp, scale=SCALE)
            else:
                nc.vector.tensor_scalar(out=eT, in0=s,
                                        scalar1=A_S, scalar2=B_S,
                                        op0=ALU.mult, op1=ALU.add)

        def emit_av(av, m, j, eT):
            eb = eT.bitcast(BF16)
            for qb in range(4):
                for h in range(2):
                    nc.tensor.matmul(
                        av[:, qb * 2 + h, 0:65],
                        lhsT=eb[:, h * 512 + qb * 128:
                                h * 512 + (qb + 1) * 128],
                        rhs=vT_sb[j][:, 2 * m + h, 0:65],
                        start=(j == 0), stop=(j == NKB - 1))

        def drain_a(av):
            """recip + normalize into avn.  DVE is free of exps at the
            pair boundary (j0/j1 go to ACT), so recip + the qb 0/1 norms
            run immediately on DVE; qb 2/3 norms queue on ACT after its
            j0/j1 exps.  drain_b transposes are split to match."""
            rden = drp.tile([128, 8, 1], F32, tag="rden", name="rden",
                            bufs=2)
            nc.vector.reciprocal(rden, av[:, :, 64:65])
            avn = drp.tile([128, 4, 128], BF16, tag="avn", name="avn",
                           bufs=2)
            for qb in range(4):
                eng = nc.vector if qb < 2 else nc.scalar
                for h in range(2):
                    src = av[:, qb * 2 + h, 0:64]
                    dst = avn[:, qb, h * 64:(h + 1) * 64]
                    if qb < 2:
                        nc.vector.tensor_scalar(
                            out=dst, in0=src,
                            scalar1=rden[:, qb * 2 + h, :],
                            scalar2=None, op0=ALU.mult)
                    else:
                        nc.scalar.activation(out=dst, in_=src, func=AF.Copy,
                                             scale=rden[:, qb * 2 + h, :])
            return avn

        def drain_b(qc, m, avn, halves=(0, 1)):
            """transpose + opair copy (+v bias), split by qb halves."""
            if 0 in halves:
                tps_flat = new_ops().bitcast(BF16)[:, 0:512]
                op = drp.tile([128, 512], BF16, tag=f"op{m}",
                              name=f"op{m}", bufs=2)
                drain_b.state = (tps_flat, op)
            tps_flat, op = drain_b.state
            tps = tps_flat.rearrange("p (qb q) -> p qb q", qb=4)
            for half in halves:
                for qb in (2 * half, 2 * half + 1):
                    nc.tensor.transpose(tps[:, qb, :], avn[:, qb, :], ident)
                nc.vector.tensor_scalar(
                    out=op[:, half * 256:(half + 1) * 256],
                    in0=tps_flat[:, half * 256:(half + 1) * 256],
                    scalar1=qkvb[:, 4 + m:5 + m],
                    scalar2=None, op0=ALU.add)
            opair[(qc, m)] = op

        def emit_outproj(qc, m2):
            ps = new_ops()
            for kc in range(MT):
                nc.tensor.matmul(
                    ps, lhsT=woutT[kc][:, m2 * 128:(m2 + 1) * 128],
                    rhs=opair[(qc, kc)],
                    start=(kc == 0), stop=(kc == MT - 1))
            yt = drp.tile([128, 512], F32, tag="yt", name="yt", bufs=4)
            if m2 % 2 == 0:
                nc.scalar.activation(out=yt, in_=ps, func=AF.Copy)
            else:
                nc.vector.tensor_copy(yt, ps)
            nc.sync.dma_start(
                out=y_out[m2 * 128:(m2 + 1) * 128,
                          qc * 512:(qc + 1) * 512],
                in_=yt)

        # One continuous software-pipelined stream over all pairs: at
        # stream slot i we emit scores+exp for stream[i] and the AV for
        # stream[i-2] (which may belong to the previous pair), so the
        # pair-boundary exp latency hides behind the next pair's scores.
        stream = [(qc, m, j) for qc, m in pairs for j in range(NKB)]
        avs = {}      # pair -> av psum tile
        eTs = {}      # (pair, j) -> eT tile
        avn_pend = None
        pending_op = None
        opj = {7: 0, 9: 1, 11: 2, 13: 3}
        for i, (qc, m, j) in enumerate(stream):
            s = emit_scores(qc, m, j)
            eT = expp.tile([128, 1024], I16, tag="eT", name="eT",
                           bufs=4)
            emit_exp(s, eT, j)
            eTs[(qc, m, j)] = eT
            if i >= 2:
                pqc, pm, pj = stream[i - 2]
                if (pqc, pm) not in avs:
                    avs[(pqc, pm)] = new_av()
                emit_av(avs[(pqc, pm)], pm, pj, eTs.pop((pqc, pm, pj)))
                if pj == NKB - 1:
                    # previous pair fully accumulated: drain it
                    avn_pend = (pqc, pm, drain_a(avs.pop((pqc, pm))))
            if j == 4 and avn_pend is not None:
                drain_b(avn_pend[0], avn_pend[1], avn_pend[2], halves=(0,))
            if j == 5 and avn_pend is not None:
                dqc, dm, avn = avn_pend
                drain_b(dqc, dm, avn, halves=(1,))
                avn_pend = None
                if dm == 1:
                    pending_op = dqc
            if pending_op is not None and j in opj:
                emit_outproj(pending_op, opj[j])
                if j == max(opj):
                    pending_op = None
        # tail: last two AVs, drain last pair, final outproj
        for i in (len(stream) - 2, len(stream) - 1):
            pqc, pm, pj = stream[i]
            if (pqc, pm) not in avs:
                avs[(pqc, pm)] = new_av()
            emit_av(avs[(pqc, pm)], pm, pj, eTs.pop((pqc, pm, pj)))
        dqc, dm, avn = (stream[-1][0], stream[-1][1],
                        drain_a(avs.pop((stream[-1][0], stream[-1][1]))))
        drain_b(dqc, dm, avn)
        for m2 in range(NT):
            emit_outproj(QC - 1, m2)

    nc.compile()
    return nc


_NC = None


def _get_nc():
    global _NC
    if _NC is None:
        _NC = _build()
    return _NC


def _gblk():
    g = np.zeros((128, 8), dtype=np.float32)
    for p in range(128):
        g[p, p // CPG] = 1.0
    return g


def kernel(x, gn_gamma, gn_beta, w_qkv, w_out, b_out, trace=False):
    x = np.asarray(x, dtype=np.float32)
    w_qkv = np.asarray(w_qkv, np.float32)
    w_out = np.asarray(w_out, np.float32)
    gblk = _gblk()
    gbt = np.ascontiguousarray(gblk.T)
    gamma = np.asarray(gn_gamma, np.float32).reshape(C)
    beta = np.asarray(gn_beta, np.float32).reshape(C)
    gbo4 = np.zeros((128, 8), dtype=np.float32)
    for t in range(4):
        gbo4[:, 2 * t] = gamma[t * 128:(t + 1) * 128]
        gbo4[:, 2 * t + 1] = beta[t * 128:(t + 1) * 128]
    ident = np.eye(128, dtype=np.float32).astype(ml_dtypes.bfloat16)

    nc = _get_nc()
    in_maps = []
    for core in range(8):
        b, hg = core // 2, core % 2
        # wqkvT cols: [q m0, q m1, k m0, k m1, v m0, v m1] for local heads
        rows = np.concatenate([
            w_qkv[hg * 256:(hg + 1) * 256, :],
            w_qkv[C + hg * 256:C + (hg + 1) * 256, :],
            w_qkv[2 * C + hg * 256:2 * C + (hg + 1) * 256, :]], axis=0)
        wqkvT = np.ascontiguousarray(rows.T).astype(ml_dtypes.bfloat16)
        woutT = np.ascontiguousarray(
            w_out[:, hg * 256:(hg + 1) * 256].T).astype(ml_dtypes.bfloat16)
        in_maps.append({
            "x": np.ascontiguousarray(x[b]).astype(ml_dtypes.bfloat16),
            "wqkvT": wqkvT,
            "woutT": woutT,
            "gbo": gbo4,
            "gblk": gblk,
            "gbt": gbt,
            "ident": ident,
        })
    res = run_bass_kernel_spmd(nc, in_maps, core_ids=list(range(8)),
                               trace=trace)
    y = np.empty((B, C, N), dtype=np.float32)
    bo = np.asarray(b_out, np.float32).reshape(C, 1)
    for b in range(B):
        y[b] = (res.results[2 * b]["y"] + res.results[2 * b + 1]["y"]
                + x[b] + bo)
    if trace:
        kernel.last_results = res
    return y


# revision 15
# speedup vs baseline: 1.0032x; 1.0032x over previous
"""EnhancedTemporalAttention Trainium2 kernel (v2).

Full module: GroupNorm(32) -> QKV 1x1conv -> 8-head attention (softmax) ->
out 1x1conv + bias -> +residual, on x [4, 512, 2048] fp32.

Sharding: 8 cores = (batch b = core//2) x (head-half hg = core%2).  Each
core computes GroupNorm stats + its 4 heads' Q/K/V over the full sequence,
attention for all 2048 queries, and a partial out-projection (contraction
over its 256 channels).  Host sums the two partials per batch and adds
residual + b_out exactly in fp32.

GroupNorm is folded into the QKV weights: w' = w * scale_c (per input
channel, scaled in place on Pool), plus a K=1 matvec for the bias term
which rides the projection PSUM->SBUF copies as a per-partition bias.

Attention uses transposed scores (keys on partitions, [k,q] layout); exp
splits between ACT (exact, 19/32 per pair) and DVE (Schraudolph int16
bit-trick into bf16 bits, 13/32).  AV runs with eT stationary / vT moving
so each matmul is only 65 output rows; softmax denominators ride a ones
column on vT; normalization is a per-partition multiply in [q,d] layout,
then a PE transpose (bf16) back to [c,q] for the out-projection, whose
PSUM result DMAs straight to DRAM.
"""
import sys

sys.path.insert(0, "/opt/trn_rl_repo")

import numpy as np
import ml_dtypes

import concourse.bacc as bacc
import concourse.bass as bass
import concourse.tile as tile
from concourse import mybir
from concourse.bass_utils import run_bass_kernel_spmd

F32 = mybir.dt.float32
F32R = mybir.dt.float32r
BF16 = mybir.dt.bfloat16
I16 = mybir.dt.int16

B = 4
C = 512
N = 2048
H = 8
HL = 4             # local heads per core
D = 64
G = 32             # groupnorm groups
CPG = C // G       # 16 channels per group
EPS = 1e-4
SCALE = D ** -0.5
NT = C // 128      # 4 input-channel tiles
MT = 2             # local qkv channel tiles (256 local channels)
NKB = N // 128     # 16 key blocks
QC = 4             # query chunks of 512
AF = mybir.ActivationFunctionType
ALU = mybir.AluOpType

# Schraudolph exp into bf16 bits: i16 = s*A_S + B_S, bitcast -> bf16
A_S = 184.6650085 * SCALE
B_S = 16249.1
# j's whose (whole-j, both-head) exp runs on ACT exactly; the rest run
# the Schraudolph bit-trick on DVE.
ACT_J = (0, 1, 3, 5, 7, 9, 11, 13, 15)


def _build(taps=False):
    nc = bacc.Bacc("TRN2", target_bir_lowering=False, debug=False)
    x_in = nc.dram_tensor("x", [C, N], BF16, kind="ExternalInput").ap()
    wqkvT_in = nc.dram_tensor("wqkvT", [C, 6 * 128], BF16,
                              kind="ExternalInput").ap()
    woutT_in = nc.dram_tensor("woutT", [MT * 128, C], BF16,
                              kind="ExternalInput").ap()
    gbo_in = nc.dram_tensor("gbo", [128, 8], F32, kind="ExternalInput").ap()
    gblk_in = nc.dram_tensor("gblk", [128, 8], F32, kind="ExternalInput").ap()
    gbt_in = nc.dram_tensor("gbt", [8, 128], F32, kind="ExternalInput").ap()
    id_in = nc.dram_tensor("ident", [128, 128], BF16,
                           kind="ExternalInput").ap()
    y_out = nc.dram_tensor("y", [C, N], F32, kind="ExternalOutput").ap()

    from contextlib import ExitStack
    with tile.TileContext(nc) as tc, ExitStack() as ctx:
        persist = ctx.enter_context(tc.tile_pool(name="persist", bufs=1))
        gn = ctx.enter_context(tc.tile_pool(name="gn", bufs=1))
        pspool = ctx.enter_context(tc.tile_pool(name="ps", bufs=1,
                                                space="PSUM"))
        expp = ctx.enter_context(tc.tile_pool(name="expp", bufs=1))
        drp = ctx.enter_context(tc.tile_pool(name="drp", bufs=1))

        # ---- persistent tiles ----
        X = [persist.tile([128, N], BF16, tag=f"X{t}", name=f"X{t}")
             for t in range(NT)]
        wT = [persist.tile([128, 6 * 128], BF16, tag=f"wT{kc}",
                           name=f"wT{kc}") for kc in range(NT)]
        woutT = [persist.tile([128, C], BF16, tag=f"woT{m}", name=f"woT{m}")
                 for m in range(MT)]
        q_sb = [persist.tile([128, N], BF16, tag=f"q{m}", name=f"q{m}")
                for m in range(MT)]
        k_sb = [persist.tile([128, N], BF16, tag=f"k{m}", name=f"k{m}")
                for m in range(MT)]
        vT_sb = [persist.tile([128, HL, 66], BF16, tag=f"vT{nb}",
                              name=f"vT{nb}") for nb in range(NKB)]
        ident = persist.tile([128, 128], BF16, tag="ident", name="ident")

        # PSUM rings: S 3x4KB + av 4KB = 16KB exactly; every other
        # psum user (GN, projections, transpose, out-proj) shares the
        # S ring, using a [:, 0:512] half-slot view.
        def new_S():
            return pspool.tile([128, 1024], F32, tag="S", name="S", bufs=3)

        def new_ops():
            return new_S()[:, 0:512]

        def new_av():
            return pspool.tile([128, 8, 128], F32, tag="av", name="av",
                               bufs=1)

        # ---- input loads ----
        # x: 8 chunks of [128,1024] on the HWDGE path (critical for stats);
        # misc/gbt woven in after chunk 3.  Weights + ident ride the
        # software DGE (gpsimd) so they bypass the serialized HWDGE device.
        gbo4 = gn.tile([128, 8], F32, tag="gbo4")
        gblk = gn.tile([128, 8], F32R, tag="gblk")
        gbt = gn.tile([8, 128], F32R, tag="gbt")
        for t in range(NT):
            for half in range(2):
                nc.sync.dma_start(
                    out=X[t][:, half * 1024:(half + 1) * 1024],
                    in_=x_in[t * 128:(t + 1) * 128,
                             half * 1024:(half + 1) * 1024])
        nc.sync.dma_start(out=gbo4, in_=gbo_in)
        nc.sync.dma_start(out=gblk, in_=gblk_in.bitcast(F32R))
        nc.sync.dma_start(out=gbt, in_=gbt_in.bitcast(F32R))
        for kc in range(NT):
            nc.sync.dma_start(out=wT[kc],
                              in_=wqkvT_in[kc * 128:(kc + 1) * 128, :])
        for m in range(MT):
            nc.sync.dma_start(out=woutT[m],
                              in_=woutT_in[m * 128:(m + 1) * 128, :])
        nc.sync.dma_start(out=ident, in_=id_in)
        gbo = [gbo4[:, 2 * t:2 * t + 2] for t in range(NT)]

        # ---- GroupNorm stats ----
        eps_t = gn.tile([G, 1], F32, tag="eps_t")
        nc.vector.memset(eps_t, EPS)
        sqw = gn.tile([G, 1], F32, tag="sqw")
        nc.scalar.activation(out=sqw, in_=eps_t, func=AF.Sqrt)
        mvv = []
        for t in range(NT):
            stats = gn.tile([128, 4, 6], F32, tag=f"st{t}", name=f"st{t}")
            for sg in range(4):
                nc.vector.bn_stats(out=stats[:, sg, :],
                                   in_=X[t][:, sg * 512:(sg + 1) * 512])
            mv = gn.tile([128, 2], F32, tag=f"mv{t}", name=f"mv{t}")
            nc.vector.bn_aggr(out=mv, in_=stats)
            mt = gn.tile([128, 2], F32R, tag=f"mvv{t}", name=f"mvv{t}")
            nc.vector.tensor_copy(mt[:, 0:1], mv[:, 0:1])
            nc.vector.scalar_tensor_tensor(
                out=mt[:, 1:2], in0=mv[:, 0:1], scalar=mv[:, 0:1],
                in1=mv[:, 1:2], op0=ALU.mult, op1=ALU.add)
            mvv.append(mt)
        g8ps = new_ops()
        for t in range(NT):
            nc.tensor.matmul(g8ps[0:8, t * 2:(t + 1) * 2],
                             lhsT=gblk, rhs=mvv[t],
                             start=(t == 0), stop=(t == NT - 1),
                             skip_group_check=True)
        g8 = gn.tile([8, NT, 2], F32, tag="g8")
        nc.vector.tensor_copy(g8.rearrange("p t s -> p (t s)"),
                              g8ps[0:8, 0:8])
        mean8 = gn.tile([8, NT], F32, tag="mean8")
        nc.vector.tensor_scalar_mul(mean8, g8[:, :, 0], 1.0 / CPG)
        ex28 = gn.tile([8, NT], F32, tag="ex28")
        nc.vector.tensor_scalar_mul(ex28, g8[:, :, 1], 1.0 / CPG)
        msq8 = gn.tile([8, NT], F32, tag="msq8")
        nc.vector.tensor_mul(msq8, mean8, mean8)
        var8 = gn.tile([8, NT], F32, tag="var8")
        nc.vector.tensor_tensor(out=var8, in0=ex28, in1=msq8,
                                op=ALU.subtract)
        std8 = gn.tile([8, NT], F32, tag="std8")
        nc.scalar.activation(out=std8, in_=var8, func=AF.Sqrt,
                             bias=eps_t[0:8, :])
        rstd8 = gn.tile([8, NT], F32, tag="rstd8")
        nc.vector.reciprocal(rstd8, std8)
        # preload the Exp table; chained after the real Sqrt via std8
        warm = gn.tile([8, NT], F32, tag="warm")
        nc.scalar.activation(out=warm, in_=std8, func=AF.Exp)
        mr8 = gn.tile([8, NT, 2], F32R, tag="mr8")
        nc.vector.tensor_copy(mr8[:, :, 0:1],
                              mean8.rearrange("p (t o) -> p t o", o=1))
        nc.vector.tensor_copy(mr8[:, :, 1:2],
                              rstd8.rearrange("p (t o) -> p t o", o=1))
        msps = new_ops()
        for t in range(NT):
            nc.tensor.matmul(msps[:, t * 2:(t + 1) * 2],
                             lhsT=gbt, rhs=mr8[:, t, :],
                             start=(t == 0), stop=(t == NT - 1),
                             skip_group_check=True)
        mscall = msps[:, 0:2 * NT].rearrange("p (t s) -> p t s", s=2)
        msb = gn.tile([128, NT, 2], F32, tag="msb")
        nc.vector.tensor_copy(msb.rearrange("p t s -> p (t s)"),
                              msps[:, 0:2 * NT])

        # per-channel scale_c = rstd*gamma; bias (bf16, for the matvec)
        # = beta - mean*scale_c, per-tile chains split DVE/Pool
        qkvb_ps = None
        scale_c = []
        for t in range(NT):
            eng = nc.vector if t % 2 == 0 else nc.gpsimd
            sc = gn.tile([128, 1], F32, tag=f"sc{t}", name=f"sc{t}")
            eng.tensor_mul(sc, msb[:, t, 1:2], gbo[t][:, 0:1])
            scale_c.append(sc)
            tmp = gn.tile([128, 1], F32, tag=f"tmp{t}", name=f"tmp{t}")
            eng.tensor_mul(tmp, msb[:, t, 0:1], sc)
            bb = gn.tile([128, 1], BF16, tag=f"bb{t}", name=f"bb{t}")
            eng.tensor_tensor(out=bb, in0=gbo[t][:, 1:2], in1=tmp,
                              op=ALU.subtract)
            # qkv bias matvec against RAW weights (before scaling)
            if qkvb_ps is None:
                qkvb_ps = new_ops()
            for oc in range(6):
                nc.tensor.matmul(qkvb_ps[:, 6 * t + oc:6 * t + oc + 1],
                                 lhsT=wT[t][:, oc * 128:(oc + 1) * 128],
                                 rhs=bb, start=True, stop=True,
                                 skip_group_check=True)
        # reduce the 4 kc partials: qkvb[:, oc] = sum_t qkvb_ps[:, 6t+oc]
        qkvb_all = gn.tile([128, NT, 6], F32, tag="qkvb_all")
        nc.vector.tensor_copy(qkvb_all.rearrange("p t s -> p (t s)"),
                              qkvb_ps[:, 0:24])
        qkvb01 = gn.tile([128, 6], F32, tag="qkvb01")
        nc.vector.tensor_tensor(out=qkvb01, in0=qkvb_all[:, 0, :],
                                in1=qkvb_all[:, 1, :], op=ALU.add)
        qkvb23 = gn.tile([128, 6], F32, tag="qkvb23")
        nc.vector.tensor_tensor(out=qkvb23, in0=qkvb_all[:, 2, :],
                                in1=qkvb_all[:, 3, :], op=ALU.add)
        qkvb = gn.tile([128, 6], F32, tag="qkvb")
        nc.vector.tensor_tensor(out=qkvb, in0=qkvb01, in1=qkvb23,
                                op=ALU.add)
        # scale weights in place (per input-channel partition)
        for t in range(NT):
            if t < 2:
                nc.scalar.activation(out=wT[t], in_=wT[t], func=AF.Copy,
                                     scale=scale_c[t])
            else:
                nc.gpsimd.tensor_scalar(out=wT[t], in0=wT[t],
                                        scalar1=scale_c[t], scalar2=None,
                                        op0=ALU.mult)

        # ---- projections ----
        def kq_proj(which, m, ncx, eng):
            ps = new_ops()
            col0 = (m if which == "q" else 2 + m) * 128
            for kc in range(NT):
                nc.tensor.matmul(
                    ps, lhsT=wT[kc][:, col0:col0 + 128],
                    rhs=X[kc][:, ncx * 512:(ncx + 1) * 512],
                    start=(kc == 0), stop=(kc == NT - 1))
            dst = (q_sb if which == "q" else k_sb)[m][
                :, ncx * 512:(ncx + 1) * 512]
            boff = (0 if which == "q" else 2) + m
            if eng == "act":
                nc.scalar.activation(out=dst, in_=ps, func=AF.Identity,
                                     bias=qkvb[:, boff:boff + 1])
            else:
                nc.vector.tensor_scalar(out=dst, in0=ps,
                                        scalar1=qkvb[:, boff:boff + 1],
                                        scalar2=None, op0=ALU.add)

        def v_proj(nb, eng):
            ps = new_ops()
            for kc in range(NT):
                nc.tensor.matmul(
                    ps[:, 0:256], lhsT=X[kc][:, nb * 128:(nb + 1) * 128],
                    rhs=wT[kc][:, 4 * 128:6 * 128],
                    start=(kc == 0), stop=(kc == NT - 1))
            src = ps[:, 0:256].rearrange("p (h d) -> p h d", h=HL)
            dst = vT_sb[nb][:, :, 0:64]
            # v bias is folded at the opair copy; here plain convert
            if eng == "act":
                nc.scalar.activation(out=dst, in_=src, func=AF.Copy)
            else:
                nc.vector.tensor_copy(dst, src)
            nc.gpsimd.memset(vT_sb[nb][:, :, 64:65], 1.0)

        engs = ["act", "dve"]
        for m in range(MT):
            for ncx in range(4):
                kq_proj("k", m, ncx, engs[(m * 4 + ncx) % 2])
        for m in range(MT):
            for ncx in range(4):
                kq_proj("q", m, ncx, engs[(m * 4 + ncx + 1) % 2])
        for nb in range(NKB):
            v_proj(nb, engs[nb % 2])

        # ---- attention ----
        pairs = [(qc, m) for qc in range(QC) for m in range(MT)]
        opair = {}

        def emit_scores(qc, m, j):
            s = new_S()
            nc.tensor.matmul(
                s[:, 0:512], lhsT=k_sb[m][0:64, j * 128:(j + 1) * 128],
                rhs=q_sb[m][0:64, qc * 512:(qc + 1) * 512],
                start=True, stop=True, tile_position=(0, 0),
                skip_group_check=True)
            nc.tensor.matmul(
                s[:, 512:1024], lhsT=k_sb[m][64:128, j * 128:(j + 1) * 128],
                rhs=q_sb[m][64:128, qc * 512:(qc + 1) * 512],
                start=True, stop=True, tile_position=(64, 0),
                skip_group_check=True)
            return s

        def emit_exp(s, eT, j):
            if j in ACT_J:
                nc.scalar.activation(out=eT.bitcast(BF16), in_=s,
                                     func=AF.Exp, scale=SCALE)
            else:
                nc.vector.tensor_scalar(out=eT, in0=s,
                                        scalar1=A_S, scalar2=B_S,
                                        op0=ALU.mult, op1=ALU.add)

        def emit_av(av, m, j, eT):
            eb = eT.bitcast(BF16)
            for qb in range(4):
                for h in range(2):
                    nc.tensor.matmul(
                        av[:, qb * 2 + h, 0:65],
                        lhsT=eb[:, h * 512 + qb * 128:
                                h * 512 + (qb + 1) * 128],
                        rhs=vT_sb[j][:, 2 * m + h, 0:65],
                        start=(j == 0), stop=(j == NKB - 1))

        def drain_a(av):
            """recip + normalize into avn.  DVE is free of exps at the
            pair boundary (j0/j1 go to ACT), so recip + the qb 0/1 norms
            run immediately on DVE; qb 2/3 norms queue on ACT after its
            j0/j1 exps.  drain_b transposes are split to match."""
            rden = drp.tile([128, 8, 1], F32, tag="rden", name="rden",
                            bufs=2)
            nc.vector.reciprocal(rden, av[:, :, 64:65])
            avn = drp.tile([128, 4, 128], BF16, tag="avn", name="avn",
                           bufs=2)
            for qb in range(4):
                eng = nc.vector if qb < 2 else nc.scalar
                for h in range(2):
                    src = av[:, qb * 2 + h, 0:64]
                    dst = avn[:, qb, h * 64:(h + 1) * 64]
                    if qb < 2:
                        nc.vector.tensor_scalar(
                            out=dst, in0=src,
                            scalar1=rden[:, qb * 2 + h, :],
                            scalar2=None, op0=ALU.mult)
                    else:
                        nc.scalar.activation(out=dst, in_=src, func=AF.Copy,
                                             scale=rden[:, qb * 2 + h, :])
            return avn

        def drain_b(qc, m, avn, halves=(0, 1)):
            """transpose + opair copy (+v bias), split by qb halves."""
            if 0 in halves:
                tps_flat = new_ops().bitcast(BF16)[:, 0:512]
                op = drp.tile([128, 512], BF16, tag=f"op{m}",
                              name=f"op{m}", bufs=2)
                drain_b.state = (tps_flat, op)
            tps_flat, op = drain_b.state
            tps = tps_flat.rearrange("p (qb q) -> p qb q", qb=4)
            for half in halves:
                for qb in (2 * half, 2 * half + 1):
                    nc.tensor.transpose(tps[:, qb, :], avn[:, qb, :], ident)
                nc.vector.tensor_scalar(
                    out=op[:, half * 256:(half + 1) * 256],
                    in0=tps_flat[:, half * 256:(half + 1) * 256],
                    scalar1=qkvb[:, 4 + m:5 + m],
                    scalar2=None, op0=ALU.add)
            opair[(qc, m)] = op

        def emit_outproj(qc, m2):
            ps = new_ops()
            for kc in range(MT):
                nc.tensor.matmul(
                    ps, lhsT=woutT[kc][:, m2 * 128:(m2 + 1) * 128],
                    rhs=opair[(qc, kc)],
                    start=(kc == 0), stop=(kc == MT - 1))
            yt = drp.tile([128, 512], F32, tag="yt", name="yt", bufs=4)
            if m2 % 2 == 0:
                nc.scalar.activation(out=yt, in_=ps, func=AF.Copy)
            else:
                nc.vector.tensor_copy(yt, ps)
            nc.sync.dma_start(
                out=y_out[m2 * 128:(m2 + 1) * 128,
                          qc * 512:(qc + 1) * 512],
                in_=yt)

        # One continuous software-pipelined stream over all pairs: at
        # stream slot i we emit scores+exp for stream[i] and the AV for
        # stream[i-2] (which may belong to the previous pair), so the
        # pair-boundary exp latency hides behind the next pair's scores.
        stream = [(qc, m, j) for qc, m in pairs for j in range(NKB)]
        avs = {}      # pair -> av psum tile
        eTs = {}      # (pair, j) -> eT tile
        avn_pend = None
        pending_op = None
        opj = {6: 0, 8: 1, 10: 2, 12: 3}
        for i, (qc, m, j) in enumerate(stream):
            s = emit_scores(qc, m, j)
            eT = expp.tile([128, 1024], I16, tag="eT", name="eT",
                           bufs=4)
            emit_exp(s, eT, j)
            eTs[(qc, m, j)] = eT
            if i >= 2:
                pqc, pm, pj = stream[i - 2]
                if (pqc, pm) not in avs:
                    avs[(pqc, pm)] = new_av()
                emit_av(avs[(pqc, pm)], pm, pj, eTs.pop((pqc, pm, pj)))
                if pj == NKB - 1:
                    # previous pair fully accumulated: drain it
                    avn_pend = (pqc, pm, drain_a(avs.pop((pqc, pm))))
            if j == 3 and avn_pend is not None:
                drain_b(avn_pend[0], avn_pend[1], avn_pend[2], halves=(0,))
            if j == 4 and avn_pend is not None:
                dqc, dm, avn = avn_pend
                drain_b(dqc, dm, avn, halves=(1,))
                avn_pend = None
                if dm == 1:
                    pending_op = dqc
            if pending_op is not None and j in opj:
                emit_outproj(pending_op, opj[j])
                if j == max(opj):
                    pending_op = None
        # tail: last two AVs, drain last pair, final outproj
        for i in (len(stream) - 2, len(stream) - 1):
            pqc, pm, pj = stream[i]
            if (pqc, pm) not in avs:
                avs[(pqc, pm)] = new_av()
            emit_av(avs[(pqc, pm)], pm, pj, eTs.pop((pqc, pm, pj)))
        dqc, dm, avn = (stream[-1][0], stream[-1][1],
                        drain_a(avs.pop((stream[-1][0], stream[-1][1]))))
        drain_b(dqc, dm, avn)
        for m2 in range(NT):
            emit_outproj(QC - 1, m2)

    nc.compile()
    return nc


_NC = None


def _get_nc():
    global _NC
    if _NC is None:
        _NC = _build()
    return _NC


def _gblk():
    g = np.zeros((128, 8), dtype=np.float32)
    for p in range(128):
        g[p, p // CPG] = 1.0
    return g


def kernel(x, gn_gamma, gn_beta, w_qkv, w_out, b_out, trace=False):
    x = np.asarray(x, dtype=np.float32)
    w_qkv = np.asarray(w_qkv, np.float32)
    w_out = np.asarray(w_out, np.float32)
    gblk = _gblk()
    gbt = np.ascontiguousarray(gblk.T)
    gamma = np.asarray(gn_gamma, np.float32).reshape(C)
    beta = np.asarray(gn_beta, np.float32).reshape(C)
    gbo4 = np.zeros((128, 8), dtype=np.float32)
    for t in range(4):
        gbo4[:, 2 * t] = gamma[t * 128:(t + 1) * 128]
        gbo4[:, 2 * t + 1] = beta[t * 128:(t + 1) * 128]
    ident = np.eye(128, dtype=np.float32).astype(ml_dtypes.bfloat16)

    nc = _get_nc()
    in_maps = []
    for core in range(8):
        b, hg = core // 2, core % 2
        # wqkvT cols: [q m0, q m1, k m0, k m1, v m0, v m1] for local heads
        rows = np.concatenate([
            w_qkv[hg * 256:(hg + 1) * 256, :],
            w_qkv[C + hg * 256:C + (hg + 1) * 256, :],
            w_qkv[2 * C + hg * 256:2 * C + (hg + 1) * 256, :]], axis=0)
        wqkvT = np.ascontiguousarray(rows.T).astype(ml_dtypes.bfloat16)
        woutT = np.ascontiguousarray(
            w_out[:, hg * 256:(hg + 1) * 256].T).astype(ml_dtypes.bfloat16)
        in_maps.append({
            "x": np.ascontiguousarray(x[b]).astype(ml_dtypes.bfloat16),
            "wqkvT": wqkvT,
            "woutT": woutT,
            "gbo": gbo4,
            "gblk": gblk,
            "gbt": gbt,
            "ident": ident,
        })
    res = run_bass_kernel_spmd(nc, in_maps, core_ids=list(range(8)),
                               trace=trace)
    y = np.empty((B, C, N), dtype=np.float32)
    bo = np.asarray(b_out, np.float32).reshape(C, 1)
    for b in range(B):
        y[b] = (res.results[2 * b]["y"] + res.results[2 * b + 1]["y"]
                + x[b] + bo)
    if trace:
        kernel.last_results = res
    return y
